# revision 43
# baseline (speedup 1.0000x reference)
"""BertAttention (T5-style relative-position bias) Trainium2 Bass kernel.

Strategy (8-way tensor parallel over heads, 2 heads/core), v5:
  - Q/K projection and QK^T scores run fp8e4m3 DoubleRow (0.5 cyc/row): the
    host packs hidden^T into the [64,2,..] DR fold (hP) and pre-scales the
    weights (Q x2, K x16) so fp8 stays in e4m3's normal range; scores land
    x256 in PSUM (exp scale 1/256) and the T5 Toeplitz bias flip-matmuls use
    x256 fp8 tables. The V projection, P@V and the output dense stay bf16 --
    quantization noise on activations feeding random matmuls passes through
    to the output at full strength, and V/ctx/dense in fp8 blew the 2e-2
    error budget (measured 2.9e-2 all-fp8 vs 1.56e-2 in this mix).
  - V is produced token-major directly by the PE from a bf16 copy of
    hidden^T (stationary = hidden tile, moving = V weights), killing the v3
    transposes; the V tile carries a 1.0 ones column against x16 V weights,
    so PV accumulates the softmax denominator and the normalized ctx comes
    out x16 (safely in range for downstream bf16).
  - exp is split across engines per score tile (ENG): ACT runs the real exp
    (bf16 out); DVE runs a Schraudolph fast-exp -- one tensor_scalar
    emitting int16 = the bf16 BITS of exp (A*s+B, rounds on HW), bitcast to
    bf16 for PV. sigma=0.0573 zeroes the mean log-ratio; the softmax ratio
    cancels most of the rest (rms 1.8%/element, which the error budget
    absorbs). GPSIMD is kept free: AllToAll collectives block the issuing
    (Pool) queue for their whole ~21us cost-model duration.
  - Scores per (batch, head, 1024-wide q chunk): s^T[k, q] in PSUM [128,1024];
    saturated-bucket constant added free via the exp bias operand (ACT) or
    the Schraudolph additive constant (DVE); only the diagonal band pays an
    anti-diagonal (flip) bias matmul. The two per-side Toeplitz windows are
    fetched with one banded DMA each per unit (shared z-base), not per-tile.
  - Softmax denominator: reciprocal (bf16) on DVE; the partition broadcast
    runs on GPSIMD for batch 0 but via two tiny PE matmuls into the drained
    ctx PSUM rows for batch 1 (whole-tail-on-Pool for b0, since a blocked
    Pool must never gate the DVE queue that carries the next unit's exps).
  - Per (batch, head) 256KB bf16 AllToAll reshards head-split -> token-split
    as soon as each head finishes, overlapping the remaining attention; the
    dense (bf16) lives in the same tile-pool scope (no phase barrier), its
    PSUM accumulators are slices of the score-tile ring (one group per 2KB
    PSUM bank), and batch 1 runs its first three passes' s0-halves before
    the final collective so only the s1-halves wait on it.
  - hidden chunks are double-prefetched two chunks ahead on SP+ACT queues;
    projection work is woven between attention k-tiles in emission order to
    keep the in-order PE queue dense while collectives overlap attention.
"""
import sys
import math

sys.path.insert(0, "/opt/trn_rl_repo")

import numpy as np
import ml_dtypes

import concourse.bass as bass
import concourse.bacc as bacc
import concourse.tile as tile
import concourse.mybir as mybir
from concourse.bass_utils import run_bass_kernel_spmd

F32 = mybir.dt.float32
BF16 = mybir.dt.bfloat16
FP8 = mybir.dt.float8e4
I16 = mybir.dt.int16
FP8_NP = mybir.dt.np(mybir.dt.float8e4)
DR = mybir.MatmulPerfMode.DoubleRow
Exp = mybir.ActivationFunctionType.Exp
ADD = mybir.AluOpType.add
MULT = mybir.AluOpType.mult

B, S, HID = 2, 2048, 1024
NH, HD = 16, 64
NB, MAXD = 32, 128
N_CORES = 8
HPC = NH // N_CORES          # heads per core = 2
T = B * S                    # 4096 flat tokens
KTILES = S // 128            # 16 k tiles per batch
QCH = 2                      # 1024-wide q chunks per batch
TW = 4096                    # padded width of expanded bias table
HB = S // N_CORES            # 256 tokens per core per batch
SAT = 91                     # |k-q| >= SAT -> bucket saturates (31 pos / 15 neg)

LOG2E = 1.4426950408889634
SIGMA = 0.0573               # Schraudolph shift (zero mean log-ratio)
SC = 1.0 / 256.0             # PSUM scores are 256x the reference scores
A16 = 128.0 * LOG2E * SC     # Schraudolph multiplier (bf16 bits via int16)
AQ, AK, AV = 2.0, 16.0, 16.0  # weight pre-scales
WDS = 64.0                   # dense weight pre-scale

# exp engine per k-tile: A=ACT real exp, D=DVE fast-exp, P=GPSIMD fast-exp
ENG = "AAAADAADAADAADAD"


def _bucket_map_rev():
    """rev[z] = bucket(2047 - z) for z in [0, 4094], T5 bidirectional buckets."""
    rel = (2047 - np.arange(TW - 1)).astype(np.int64)   # k - q
    nb = NB // 2                                        # 16
    base = np.where(rel > 0, nb, 0)
    r = np.abs(rel)
    max_exact = nb // 2                                 # 8
    is_small = r < max_exact
    tmp = np.log(np.maximum(r, 1).astype(np.float32) / np.float32(max_exact))
    large = tmp / np.float32(math.log(MAXD / max_exact)) * np.float32(nb - max_exact)
    large_i = max_exact + large.astype(np.int32)
    large_i = np.minimum(large_i, nb - 1)
    return (base + np.where(is_small, r, large_i)).astype(np.int32)  # [4095]


def _tile_plan(kt, qch):
    """(side, w_lo, w_hi) for a [128k x 1024q] score tile.

    side 0: activation adds c_pos, side 1: adds c_neg. [w_lo, w_hi) is the
    column range (may be empty) where bias - c_side is nonzero; it is covered
    by a flip-matmul against the side-shifted Toeplitz table.
    """
    q0 = qch * 1024
    k0 = kt * 128
    b_lo = max(k0 - (SAT - 1) - q0, 0)           # cols < b_lo: all k-q >= SAT
    b_hi = min(k0 + 127 + (SAT - 1) - q0 + 1, 1024)  # cols >= b_hi: all k-q <= -SAT
    if b_lo >= 1024:       # fully pos-saturated
        return 0, 0, 0
    if b_hi <= 0:          # fully neg-saturated
        return 1, 0, 0
    left, right = b_lo, 1024 - b_hi
    if left >= right:      # leave left (pos side) to the constant
        return 0, b_lo, 1024
    return 1, 0, b_hi


def _fold8(w):
    """[HID, F] -> DR fold [64, 8, 2, F]: out[kp, kt, i, f] = w[128kt+64i+kp, f]."""
    F = w.shape[1]
    return np.ascontiguousarray(
        w.reshape(8, 2, 64, F).transpose(2, 0, 1, 3)).astype(FP8_NP)


def _build_program():
    nc = bacc.Bacc("TRN2", target_bir_lowering=False, debug=False,
                   enable_asserts=True, num_devices=N_CORES)

    hP_d = nc.dram_tensor("hP", [64, 8, 2, T], FP8, kind="ExternalInput")
    wqk_d = nc.dram_tensor("wqk", [64, 8, 2, 256], FP8, kind="ExternalInput")
    hB_d = nc.dram_tensor("hB", [128, 8, T], BF16, kind="ExternalInput")
    wv_d = nc.dram_tensor("wv", [128, 8, 128], BF16, kind="ExternalInput")
    bq_d = nc.dram_tensor("bq", [128, 1], F32, kind="ExternalInput")
    wd_d = nc.dram_tensor("wd", [128, 8, HID], BF16, kind="ExternalInput")
    bd_d = nc.dram_tensor("bd", [HID, 1], F32, kind="ExternalInput")
    cv_d = nc.dram_tensor("cv", [128, 4], F32, kind="ExternalInput")
    cs_d = nc.dram_tensor("cs", [128, 4], F32, kind="ExternalInput")
    tv_d = nc.dram_tensor("tv", [HPC, 2, TW], FP8, kind="ExternalInput")
    jp_d = nc.dram_tensor("jp", [64, 2, 128], FP8, kind="ExternalInput")
    out_d = nc.dram_tensor("outT", [HID, T // N_CORES], F32, kind="ExternalOutput")

    with tile.TileContext(nc) as tc:
        with tc.tile_pool(name="const", bufs=1) as cst, \
             tc.tile_pool(name="big", bufs=1) as big, \
             tc.tile_pool(name="dram", bufs=1, space="DRAM") as dram:

            # ---------------- constants / weights ----------------
            wqk_sb = cst.tile([64, 8, 2, 256], FP8, tag="wqk")
            wv_sb = cst.tile([128, 8, 128], BF16, tag="wv")
            bq_sb = cst.tile([128, 1], F32, tag="bq")
            bd_sb = cst.tile([128, 8, 1], F32, tag="bd")
            cv_sb = cst.tile([128, 4], F32, tag="cv")
            cs_sb = cst.tile([128, 4], F32, tag="cs")
            wd_sb = big.tile([128, 8, HID], BF16, tag="wd")
            nc.gpsimd.dma_start(wd_sb[:], wd_d[:, :, :])
            jp_sb = cst.tile([64, 2, 128], FP8, tag="jp")
            nc.sync.dma_start(jp_sb[:], jp_d[:, :, :])
            ones64 = cst.tile([1, 64], BF16, tag="ones64")
            nc.gpsimd.memset(ones64[:], 1.0)

            # packed fp8 Q/K for DoubleRow QK: [64 = 2 heads x 32, 2 k-subtiles, S]
            QPb = [big.tile([64, 2, S], FP8, tag=f"QP{b}", name=f"QP{b}") for b in range(B)]
            KPb = [big.tile([64, 2, S], FP8, tag=f"KP{b}", name=f"KP{b}") for b in range(B)]
            # V token-major bf16: [128 k, kt, 2 heads x (64 feats + 16.0 col)]
            Vaugb = [big.tile([128, KTILES, 130], BF16, tag=f"Vaug{b}", name=f"Vaug{b}")
                     for b in range(B)]
            ctxb = [big.tile([64, HPC, S], BF16, tag=f"ctx{b}", name=f"ctx{b}")
                    for b in range(B)]

            a2a_in = [[dram.tile([512, HB], BF16, name=f"a2ai{b}_{s}")
                       for s in range(HPC)] for b in range(B)]
            a2a_out = [[dram.tile([512, HB], BF16, name=f"a2ao{b}_{s}")
                        for s in range(HPC)] for b in range(B)]

            tvb = tv_d[:, :, :]

            with tc.tile_pool(name="htp", bufs=3) as htp, \
                 tc.tile_pool(name="rp", bufs=3) as rp, \
                 tc.tile_pool(name="expp", bufs=5) as expp, \
                 tc.tile_pool(name="nrm", bufs=4) as nrm, \
                 tc.tile_pool(name="nrm2", bufs=2) as nrm2, \
                 tc.tile_pool(name="dns", bufs=8) as dns, \
                 tc.tile_pool(name="sps", bufs=3, space="PSUM") as sps, \
                 tc.tile_pool(name="cps", bufs=1, space="PSUM") as cps:

                pending_shuf = []
                pending_norm = []

                def flush_norm():
                    while pending_norm:
                        pending_norm.pop(0)()
                ht_pre = {}

                def prefetch_ht(b, tci, engs=None):
                    gci = b * 4 + tci
                    ht = htp.tile([64, 16, 512], FP8, tag="ht",
                                  name=f"ht{b}_{tci}")
                    htb = htp.tile([128, 8, 512], BF16, tag="htb",
                                   name=f"htb{b}_{tci}")
                    hsrc = hP_d[:, :, :, 512 * gci:512 * (gci + 1)]
                    bsrc = hB_d[:, :, 512 * gci:512 * (gci + 1)]
                    e0, e1 = engs or (nc.sync, nc.scalar)
                    b0e, b1e = (nc.gpsimd, nc.gpsimd) if engs else (e0, e1)
                    e0.dma_start(
                        ht[:].rearrange("p (kt i) t -> p kt i t", i=2)[:, 0:4],
                        hsrc[:, 0:4])
                    e1.dma_start(
                        ht[:].rearrange("p (kt i) t -> p kt i t", i=2)[:, 4:8],
                        hsrc[:, 4:8])
                    b0e.dma_start(htb[:, 0:4, :], bsrc[:, 0:4])
                    b1e.dma_start(htb[:, 4:8, :], bsrc[:, 4:8])
                    ht_pre[(b, tci)] = (ht, htb)

                def fetch_ht(b, tci):
                    if (b, tci) in ht_pre:
                        return ht_pre.pop((b, tci))
                    prefetch_ht(b, tci)
                    return ht_pre.pop((b, tci))

                def flush_shuf():
                    while pending_shuf:
                        pending_shuf.pop(0)()

                def qkv_chunk_pieces(b, tci, eager_eng=None):
                    """Emission pieces so projection work can weave between
                    attention kt tiles. One chunk = 512 tokens."""
                    state = {}
                    gci = b * 4 + tci
                    first = b == 0 and tci == 0

                    def piece_a():
                        if tci == 0:
                            # ones column 1.0 while V is x16: the normalize
                            # then yields 16*ctx, keeping ctx fp8 in e4m3's
                            # normal range; undone in the dense output scale
                            nc.gpsimd.memset(Vaugb[b][:, :, 64:65], 1.0)
                            nc.gpsimd.memset(Vaugb[b][:, :, 129:130], 1.0)
                        if first:
                            nc.scalar.dma_start(wv_sb[:], wv_d[:, :, :])
                            nc.scalar.dma_start(bq_sb[:], bq_d[:, :])
                            nc.scalar.dma_start(cv_sb[:], cv_d[:, :])
                            nc.scalar.dma_start(cs_sb[:], cs_d[:, :])
                            nc.scalar.dma_start(
                                bd_sb[:], bd_d[:, :].rearrange("(e p) o -> p e o", p=128))
                        ht, htb = fetch_ht(b, tci)
                        state["ht"] = ht
                        state["htb"] = htb
                        if tci + 2 <= 3:
                            prefetch_ht(b, tci + 2)
                        if b == 0 and tci == 3:
                            prefetch_ht(1, 0)
                            prefetch_ht(1, 1)
                        flush_shuf()
                        pa = sps.tile([128, 1024], F32, tag="S")
                        state["pa"] = pa
                        for kt in range(8):
                            nc.tensor.matmul(pa[:, 0:512],
                                             wqk_sb[:, kt, :, 0:128],
                                             ht[:, 2 * kt:2 * kt + 2, :],
                                             start=(kt == 0), stop=(kt == 7),
                                             perf_mode=DR)

                    def piece_b():
                        pa, ht = state["pa"], state["ht"]
                        for kt in range(8):
                            nc.tensor.matmul(pa[:, 512:1024],
                                             wqk_sb[:, kt, :, 128:256],
                                             ht[:, 2 * kt:2 * kt + 2, :],
                                             start=(kt == 0), stop=(kt == 7),
                                             perf_mode=DR)
                        qst = nrm.tile([128, 512], FP8, tag="qst")
                        kst = nrm.tile([128, 512], FP8, tag="kst")
                        state["qk"] = (qst, kst)
                        nc.vector.tensor_tensor(
                            qst[:], pa[:, 0:512],
                            bq_sb[:, 0:1].to_broadcast([128, 512]), ADD)
                        nc.vector.tensor_copy(kst[:], pa[:, 512:1024])
                        cols = slice(512 * tci, 512 * (tci + 1))

                        def do_shuf(b=b, cols=cols, qst=qst, kst=kst):
                            eng = eager_eng or nc.sync
                            for s in range(HPC):
                                for i in range(2):
                                    o = 64 * s + 32 * i
                                    eng.dma_start(
                                        QPb[b][32 * s:32 * s + 32, i, cols],
                                        qst[o:o + 32, :])
                                    eng.dma_start(
                                        KPb[b][32 * s:32 * s + 32, i, cols],
                                        kst[o:o + 32, :])
                        if eager_eng is not None:
                            do_shuf()
                        else:
                            pending_shuf.append(do_shuf)

                    def piece_c():
                        htb = state["htb"]
                        pb = sps.tile([128, 1024], F32, tag="S")
                        state["pb"] = pb
                        for t4 in range(4):
                            for kt in range(8):
                                nc.tensor.matmul(
                                    pb[:, 128 * t4:128 * (t4 + 1)],
                                    htb[:, kt, 128 * t4:128 * (t4 + 1)],
                                    wv_sb[:, kt, :],
                                    start=(kt == 0), stop=(kt == 7))

                    def piece_d():
                        pb = state["pb"]
                        for t4 in range(4):
                            tt = 4 * tci + t4
                            nc.vector.tensor_copy(
                                Vaugb[b][:, tt, :]
                                .rearrange("p (g c) -> p g c", c=65)[:, :, 0:64],
                                pb[:, 128 * t4:128 * (t4 + 1)]
                                .rearrange("p (g c) -> p g c", c=64))
                        flush_shuf()

                    return [piece_a, piece_b, piece_c, piece_d]

                def qkv_chunk(b, tci, eager_eng=None):
                    for p in qkv_chunk_pieces(b, tci, eager_eng=eager_eng):
                        p()

                def attn_phase(b, piece_filler=None):
                    for s in range(HPC):
                        for qch in range(QCH):
                            pf = (piece_filler(2 * s + qch)
                                  if piece_filler is not None else None)
                            points, pieces = pf if pf else ((), [])
                            q0 = qch * 1024
                            plans = [_tile_plan(kt, qch) for kt in range(KTILES)]
                            # one banded Toeplitz load per side: side-0 tiles
                            # all start at z=1830; side-1 tiles share the
                            # window base z=1536 (zero-padded tails included)
                            RT = {}
                            for side, W, basez in ((0, 602, 1830),
                                                   (1, 602, 1536)):
                                rt = rp.tile([64, 2, W], FP8, tag=f"rt{side}")
                                rsrc = bass.AP(
                                    tvb.tensor,
                                    tvb.offset + (2 * s + side) * TW + basez,
                                    [[1, 64], [64, 2], [1, W]])
                                nc.sync.dma_start(rt[:], rsrc)
                                RT[side] = rt
                            ctx_ps = cps.tile([65, 1024], F32, tag="ctx",
                                              name=f"ctx{b}_{s}_{qch}")

                            def emit_pv(kt, esv):
                                for eh in range(2):
                                    nc.tensor.matmul(
                                        ctx_ps[:, 512 * eh:512 * (eh + 1)],
                                        Vaugb[b][:, kt, 65 * s:65 * s + 65],
                                        esv[:, 512 * eh:512 * (eh + 1)],
                                        start=(kt == 0),
                                        stop=(kt == KTILES - 1))

                            # PV lags QK/exp by 3 tiles so the first PV of a
                            # unit (carrying the anti-dep on the single ctx
                            # PSUM buffer) never stalls the in-order PE queue.
                            pvq = []
                            for kt in range(KTILES):
                                side, w_lo, w_hi = plans[kt]
                                s_ps = sps.tile([128, 1024], F32, tag="S")
                                for qh in range(2):
                                    r0, r1 = 512 * qh, 512 * (qh + 1)
                                    has_bias = w_lo < r1 and w_hi > r0
                                    nc.tensor.matmul(
                                        s_ps[:, r0:r1],
                                        KPb[b][32 * s:32 * s + 32, :,
                                               kt * 128:kt * 128 + 128],
                                        QPb[b][32 * s:32 * s + 32, :,
                                               q0 + r0:q0 + r1],
                                        start=True, stop=not has_bias,
                                        perf_mode=DR)
                                    if has_bias:
                                        c0, c1 = max(w_lo, r0), min(w_hi, r1)
                                        zo = (0 if side == 0
                                              else 384 - 128 * (kt - 8 * qch))
                                        nc.tensor.matmul(
                                            s_ps[:, c0:c1], jp_sb[:],
                                            RT[side][:, :, zo + c0 - w_lo:
                                                     zo + c1 - w_lo],
                                            start=False, stop=True, perf_mode=DR)
                                e = ENG[kt]
                                hs = 2 * s + side
                                if e == "A":
                                    es = expp.tile([128, 1024], BF16, tag="es4")
                                    nc.scalar.activation(
                                        es[:], s_ps[:], Exp,
                                        bias=cv_sb[:, hs:hs + 1], scale=SC)
                                    esv = es[:]
                                else:
                                    eng = nc.vector if e == "D" else nc.gpsimd
                                    es = expp.tile([128, 1024], I16, tag="esi")
                                    eng.tensor_scalar(
                                        es[:], s_ps[:], A16, cs_sb[:, hs:hs + 1],
                                        MULT, ADD)
                                    esv = es[:].bitcast(BF16)
                                pvq.append((kt, esv))
                                if len(pvq) > 3:
                                    emit_pv(*pvq.pop(0))
                                if kt in points and pieces:
                                    pieces.pop(0)()
                            while pvq:
                                emit_pv(*pvq.pop(0))
                            cc = nrm2.tile([65, 1024], F32, tag="cc")
                            nc.vector.tensor_copy(cc[:], ctx_ps[:])
                            recip = nrm2.tile([1, 1024], BF16, tag="rc")
                            with nc.allow_low_precision(
                                    reason="bf16 recip: 2^-9 on denominator"):
                                nc.vector.reciprocal(recip[:], cc[64:65, :])

                            if b == 0:
                                # whole tail on Pool: it may wait behind a
                                # collective there without blocking the DVE
                                # queue (which carries the next unit's exps)
                                rbb = nrm2.tile([64, 1024], BF16, tag="rb")
                                nc.gpsimd.partition_broadcast(rbb[:], recip[:])
                                nc.gpsimd.tensor_tensor(
                                    ctxb[b][:, s, q0:q0 + 1024],
                                    cc[0:64, :], rbb[:], MULT)
                            else:
                                # Pool is collective-blocked during batch 1:
                                # broadcast via two tiny PE matmuls into the
                                # drained ctx PSUM rows instead
                                for bh in range(2):
                                    nc.tensor.matmul(
                                        ctx_ps[0:64, 512 * bh:512 * (bh + 1)],
                                        ones64[:],
                                        recip[:, 512 * bh:512 * (bh + 1)],
                                        start=True, stop=True)
                                nc.vector.tensor_tensor(
                                    ctxb[b][:, s, q0:q0 + 1024],
                                    cc[0:64, :], ctx_ps[0:64, :], MULT)
                            nc.gpsimd.dma_start(
                                a2a_in[b][s][256 * qch:256 * (qch + 1), :]
                                .rearrange("(j d) t -> d j t", d=64),
                                ctxb[b][:, s, q0:q0 + 1024]
                                .rearrange("d (j t) -> d j t", t=HB))

                        # head (b, s) staged: all-to-all overlaps the rest
                        nc.gpsimd.collective_compute(
                            "AllToAll", mybir.AluOpType.bypass,
                            replica_groups=[list(range(N_CORES))],
                            ins=[a2a_in[b][s][:].opt()],
                            outs=[a2a_out[b][s][:].opt()])

                nc.sync.dma_start(wqk_sb[:], wqk_d[:, :, :, :])
                prefetch_ht(0, 0, engs=(nc.sync, nc.scalar))
                prefetch_ht(0, 1, engs=(nc.sync, nc.scalar))
                for tci in range(2):
                    qkv_chunk(0, tci, eager_eng=nc.gpsimd)

                def f0(u):
                    if u == 0:
                        return ((1, 3, 5, 7, 8, 9, 10, 11),
                                qkv_chunk_pieces(0, 2, eager_eng=nc.gpsimd)
                                + qkv_chunk_pieces(0, 3, eager_eng=nc.gpsimd))
                    return ((2, 6, 10, 14), qkv_chunk_pieces(1, u - 1))

                def f1(u):
                    if u == 0:
                        return ((2, 4, 6, 8),
                                qkv_chunk_pieces(1, 3, eager_eng=nc.sync))
                    return None

                attn_phase(0, piece_filler=f0)
                flush_shuf()
                attn_phase(1, piece_filler=f1)

                # ---- dense (same pool scope: no phase barrier; psum
                # accumulators are slices of the score-tile ring) ----
                outT_sb = big.tile([128, 8, 2 * HB], F32, tag="outT")
                out_r = out_d[:, :].rearrange("(e p) t -> p e t", p=128)
                for b in range(B):
                    cfs = []
                    for blk in range(8):        # blk = s*4 + j
                        s, j = blk // 4, blk % 4
                        cf = dns.tile([128, HB], BF16, tag="cf",
                                      name=f"cf{b}_{blk}")
                        ceng = nc.sync if blk % 2 == 0 else nc.scalar
                        ceng.dma_start(
                            cf[:], a2a_out[b][s][128 * j:128 * (j + 1), :])
                        cfs.append(cf)
                    # 4 passes x 2 output blocks; one accumulation group per
                    # PSUM bank (zero regions are bank-granular). Passes 0-2
                    # run their s0 halves first (ready at the earlier A2A) so
                    # only the s1 halves sit behind the final collective in
                    # the in-order PE queue; pass 3 reuses pass 0's ring slot.
                    dpts = {}

                    def dense_half(p4, blks):
                        if p4 not in dpts:
                            dpts[p4] = sps.tile([128, 1024], F32, tag="S",
                                                name=f"dp{b}_{p4}")
                        dpt = dpts[p4]
                        psb = [dpt[:, 0:HB], dpt[:, 512:512 + HB]]
                        for blk in blks:
                            s, j = blk // 4, blk % 4
                            for h in range(2):
                                e = 2 * p4 + h
                                nc.tensor.matmul(
                                    psb[h],
                                    wd_sb[:, blk, 128 * e:128 * (e + 1)],
                                    cfs[blk][:], start=(blk == 0),
                                    stop=(blk == 7))
                        if blks[-1] != 7:
                            return
                        for h in range(2):
                            e = 2 * p4 + h
                            nc.vector.tensor_scalar(
                                outT_sb[:, e, b * HB:(b + 1) * HB], psb[h],
                                1.0 / 16.0, bd_sb[:, e, 0:1], MULT, ADD)
                            deng = nc.sync if h == 0 else nc.scalar
                            deng.dma_start(
                                out_r[:, e, b * HB:(b + 1) * HB],
                                outT_sb[:, e, b * HB:(b + 1) * HB])

                    s0, s1 = list(range(4)), list(range(4, 8))
                    for p4 in range(3):
                        dense_half(p4, s0)
                    for p4 in range(3):
                        dense_half(p4, s1)
                    dense_half(3, s0 + s1)

    nc.compile()
    return nc


_NC_CACHE = None


def _get_program():
    global _NC_CACHE
    if _NC_CACHE is None:
        _NC_CACHE = _build_program()
    return _NC_CACHE


def _make_inmaps(hidden_states, w_qkv, b_qkv, w_dense, b_dense, rel_attn_table):
    hidden_states = np.asarray(hidden_states, dtype=np.float32)
    w_qkv = np.asarray(w_qkv, dtype=np.float32)
    b_qkv = np.asarray(b_qkv, dtype=np.float32)
    w_dense = np.asarray(w_dense, dtype=np.float32)
    b_dense = np.asarray(b_dense, dtype=np.float32)
    rel_attn_table = np.asarray(rel_attn_table, dtype=np.float32)

    hT = np.ascontiguousarray(hidden_states.reshape(T, HID).T)   # [HID, T]
    hP = _fold8(hT)                                              # [64, 8, 2, T]
    hB = np.ascontiguousarray(
        hT.reshape(8, 128, T).transpose(1, 0, 2)).astype(ml_dtypes.bfloat16)
    bm = _bucket_map_rev()
    # packed anti-diagonal (flip) matrix for DoubleRow bias matmuls:
    # jp[k', i, m] = 1 iff 64*i + k' == 127 - m
    jp_pack = np.zeros((64, 2, 128), dtype=FP8_NP)
    for i in range(2):
        for kp in range(64):
            jp_pack[kp, i, 127 - 64 * i - kp] = FP8_NP(1.0)

    # dense contraction block blk=(s, j), partition p=(i, d) = w_dense row
    # 256j + 128i + 64s + d; bf16 [128, 8, HID].
    # V projection bias folded into the dense bias.
    wdP = np.ascontiguousarray(
        w_dense.reshape(4, 2, 2, 64, HID)
        .transpose(1, 3, 2, 0, 4).reshape(128, 8, HID)).astype(ml_dtypes.bfloat16)
    bv = b_qkv[2 * HID:3 * HID]
    bd_f = (bv @ w_dense + b_dense).astype(np.float32).reshape(HID, 1)

    in_maps = []
    for c in range(N_CORES):
        ha, hb = HPC * c, HPC * c + 1
        qcols = [w_qkv[:, h * HD:(h + 1) * HD] * AQ for h in (ha, hb)]
        kcols = [w_qkv[:, HID + h * HD:HID + (h + 1) * HD] * AK for h in (ha, hb)]
        vcols = [w_qkv[:, 2 * HID + h * HD:2 * HID + (h + 1) * HD] * AV
                 for h in (ha, hb)]
        wqkP = _fold8(np.concatenate(qcols + kcols, axis=1))     # [64,8,2,256]
        wvB = np.ascontiguousarray(
            np.concatenate(vcols, axis=1).reshape(8, 128, 128)
            .transpose(1, 0, 2)).astype(ml_dtypes.bfloat16)      # [128,8,128]
        bq_c = np.concatenate(
            [b_qkv[h * HD:(h + 1) * HD] * AQ for h in (ha, hb)]
        ).reshape(128, 1).astype(np.float32)
        # expanded bias table minus the side constant, x256: [head, side, TW]
        texp = rel_attn_table[[ha, hb]][:, bm]                   # [2, 4095]
        tv_c = np.zeros((HPC, 2, TW), dtype=FP8_NP)
        for si, bucket in ((0, 31), (1, 15)):
            tv_c[:, si, :TW - 1] = (
                (texp - rel_attn_table[[ha, hb], bucket][:, None]) / SC
            ).astype(FP8_NP)
        cvv = np.array([rel_attn_table[ha, 31], rel_attn_table[ha, 15],
                        rel_attn_table[hb, 31], rel_attn_table[hb, 15]],
                       dtype=np.float64)
        cv_c = np.tile(cvv.astype(np.float32), (128, 1))
        cs_c = np.tile((128.0 * (127.0 - SIGMA + LOG2E * cvv)).astype(np.float32),
                       (128, 1))
        in_maps.append({
            "hP": hP,
            "hB": hB,
            "wqk": wqkP,
            "wv": wvB,
            "bq": bq_c,
            "wd": wdP,
            "bd": bd_f,
            "cv": cv_c,
            "cs": cs_c,
            "tv": tv_c,
            "jp": jp_pack,
        })
    return in_maps


def kernel(hidden_states, w_qkv, b_qkv, w_dense, b_dense, rel_attn_table):
    in_maps = _make_inmaps(hidden_states, w_qkv, b_qkv, w_dense, b_dense,
                           rel_attn_table)
    nc = _get_program()
    res = run_bass_kernel_spmd(nc, in_maps, core_ids=list(range(N_CORES)))
    full = np.empty((HID, T), dtype=np.float32)
    for c in range(N_CORES):
        o = res.results[c]["outT"]            # [HID, 2*HB]: [b0 block c | b1 block c]
        full[:, c * HB:(c + 1) * HB] = o[:, :HB]
        full[:, S + c * HB:S + (c + 1) * HB] = o[:, HB:]
    return np.ascontiguousarray(full.T).reshape(B, S, HID)
